# revision 18
# baseline (speedup 1.0000x reference)
"""AMRBART VocabEmbed segment-mean kernel for 8 Trainium2 NeuronCores.

Computes, for two token streams (amr, text):
    feats = embed[token_ids]            # [B, T, D] gather
    means = segment_mean(feats, segs)   # [B, G, D] (empty groups -> 0)
    out   = concat([amr_means, text_means], axis=1)  # [B, 2G, D]

Strategy (data-parallel over batch, no collectives):
  - each of the 8 cores handles B/8 = 2 batch rows x 2 streams.
  - the host packs whole token-groups into 128-token "windows"; a bf16
    matmul onehot[128 tok x 128 slots].T @ feats[128 tok x 1024] produces
    group means directly (one-hot weight = 1/count), accumulated in PSUM
    (2 windows share a PSUM tile via slot bases 0/64).
  - each core's <=16384 tokens hit <16384 distinct vocab rows, so the host
    builds a per-core compacted bf16 table whose indices fit int16 — this
    enables the fast dma_gather/dma_scatter_add (SWDGE "Ant") instructions.
  - outputs are written DENSELY in slot order (full-rate contiguous DMA);
    the host unpermutes slot -> (stream, group) rows afterwards.  The
    compiled graph is identical across cores: all per-core variation is
    carried in input tensors.
"""

import os
import sys
from contextlib import ExitStack

sys.path.insert(0, "/opt/trn_rl_repo")

import numpy as np
import ml_dtypes

from concourse import bacc, bass, mybir
import concourse.tile as tile
from concourse.bass_utils import run_bass_kernel_spmd

BF16 = ml_dtypes.bfloat16

V, D = 50265, 1024
B, T, G = 16, 4096, 1024
NCORES = 8
P = 128                     # SBUF partitions == tokens per window
RB = B // NCORES            # batch rows per core
NRS = RB * 2                # row-streams per core (amr/text per row)
SLOT_CAP = 32               # max group-row span per window
WPP = 4                     # windows per PSUM tile (SLOT_CAP * WPP == P)
CHW = 8                     # windows per gather chunk (1024 idx per
                            # dma_gather; 2048 crashed the runtime)
SPG = 4                     # PSUM tiles per scatter group
NROWS = NRS * G             # real output rows per core
TRASH = NROWS               # extra row absorbing pad-slot zero-adds
OUT_BF16 = os.environ.get("KERNEL_OUT_F32", "") != "1"

# filled by kernel() for test harness introspection
LAST_EXEC_NS = None
LAST_TRACE = None


def _pack_streams(tok_rows, seg_rows):
    """Pack NRS row-streams (already segment-sorted) of one core into
    whole-group windows of <= P tokens and <= SLOT_CAP group-row span.

    Group-rows are global: row = rs * G + g.  Returns a list of windows
    [(tok_ids int32[ntok], cols int32[ntok], wts f32[ntok], row_lo, span)].
    """
    windows = []
    cur_tok = []
    cur_col = []
    cur_wt = []
    cur_lo = None
    cur_hi = None

    def flush():
        nonlocal cur_tok, cur_col, cur_wt, cur_lo, cur_hi
        if cur_lo is None:
            return
        span = cur_hi - cur_lo + 1
        assert span <= SLOT_CAP and len(cur_tok) <= P
        windows.append((
            np.array(cur_tok, dtype=np.int32),
            np.array(cur_col, dtype=np.int32),
            np.array(cur_wt, dtype=np.float32),
            cur_lo, span,
        ))
        cur_tok, cur_col, cur_wt, cur_lo, cur_hi = [], [], [], None, None

    for rs in range(NRS):
        tok, seg = tok_rows[rs], seg_rows[rs]
        bounds = np.flatnonzero(np.diff(seg)) + 1
        starts = np.concatenate(([0], bounds))
        ends = np.concatenate((bounds, [T]))
        counts = ends - starts
        gvals = seg[starts]
        for s, e, n, g in zip(starts, ends, counts, gvals):
            row = rs * G + int(g)
            n = int(n)
            assert n <= P, f"group with {n} > {P} tokens not packable"
            if cur_lo is not None and (
                len(cur_tok) + n > P or row - cur_lo + 1 > SLOT_CAP
            ):
                flush()
            if cur_lo is None:
                cur_lo = row
            cur_hi = row
            w = 1.0 / n
            cur_tok.extend(tok[s:e].tolist())
            col = row - cur_lo
            cur_col.extend([col] * n)
            cur_wt.extend([w] * n)
    flush()
    return windows


def _prepare_core(tok_rows, seg_rows):
    return _pack_streams(tok_rows, seg_rows)


def _wrap_idx(flat):
    """dma_gather/dma_scatter_add index layout: flat index i lives at
    partition i%16, column i//16; the 16-partition block is replicated to
    all 128 partitions.  len(flat) must be a multiple of 16."""
    n = len(flat)
    assert n % 16 == 0
    f = np.asarray(flat, dtype=np.int16).reshape(n // 16, 16).T  # [16, n/16]
    return np.ascontiguousarray(np.tile(f, (8, 1)))              # [128, n/16]


def _core_tensors(windows, nwin, npsum):
    """Build per-core input tensors: compact-id gather idx (wrapped),
    one-hot weights, and the slot -> output-row map used by the host to
    unpermute the densely written output."""
    # flat gather list: slot i = w*128 + p  -> token id (0 pad)
    gflat = np.zeros(nwin * P, dtype=np.int64)
    oh = np.zeros((P, nwin, P), dtype=BF16)
    # flat slot->row map: slot i = k*128 + p -> output row (TRASH pad)
    sflat = np.full(npsum * P, TRASH, dtype=np.int64)

    for w, (tids, cols, wts, row_lo, span) in enumerate(windows):
        ntok = len(tids)
        gflat[w * P:w * P + ntok] = tids
        sub = w % WPP
        base = sub * SLOT_CAP
        oh[np.arange(ntok), w, base + cols] = wts.astype(BF16)
        k = w // WPP
        sflat[k * P + base:k * P + base + span] = row_lo + np.arange(span)
    return gflat, oh, sflat


def _build_graph(nwin, chunk_sizes, group_sizes, nu):
    npsum = nwin // WPP
    out_dt = mybir.dt.bfloat16 if OUT_BF16 else mybir.dt.float32

    nc = bacc.Bacc()
    table_p = nc.declare_dram_parameter("table", [nu, D], mybir.dt.bfloat16, False)
    gidx_p = nc.declare_dram_parameter("gidx", [P, nwin * 8], mybir.dt.int16, False)
    oh_p = nc.declare_dram_parameter("oh", [P, nwin, P], mybir.dt.bfloat16, False)
    # dense slot-ordered output: slot (k*128 + p) lives at out[p, k, :]
    out_p = nc.declare_dram_parameter("out", [P, npsum, D], out_dt, True)

    with ExitStack() as ctx:
        tc = ctx.enter_context(tile.TileContext(nc))
        const_pool = ctx.enter_context(tc.tile_pool(name="const", bufs=1))
        feat_pool = ctx.enter_context(tc.tile_pool(name="feats", bufs=3))
        psum_pool = ctx.enter_context(tc.tile_pool(name="psum", bufs=3, space="PSUM"))
        stage_pool = ctx.enter_context(tc.tile_pool(name="stage", bufs=3))

        # gidx loads first: the first gather only needs it (tiny), while the
        # one-hot load (4.5 MB) overlaps the first gather streams.
        gidx_sb = const_pool.tile([P, nwin * 8], mybir.dt.int16)
        nc.sync.dma_start(out=gidx_sb[:], in_=gidx_p[:, :])
        oh_sb = const_pool.tile([P, nwin * P], mybir.dt.bfloat16)
        nc.sync.dma_start(out=oh_sb[:], in_=oh_p.ap().rearrange("p w q -> p (w q)"))

        psum_t = None
        stage_t = None
        sct_idx = 0
        sct_fill = 0
        copy_engine = 0

        for ch, csz in enumerate(chunk_sizes):
            w0 = sum(chunk_sizes[:ch])
            feats = feat_pool.tile([P, csz, D], mybir.dt.bfloat16)
            nc.gpsimd.dma_gather(
                out_ap=feats[:],
                in_ap=table_p[:, :],
                idxs_ap=gidx_sb[:, w0 * 8:(w0 + csz) * 8],
                num_idxs=csz * P,
                num_idxs_reg=csz * P,
                elem_size=D,
            )
            for wi in range(csz):
                w = w0 + wi
                sub = w % WPP
                if sub == 0:
                    psum_t = psum_pool.tile([P, D], mybir.dt.float32)
                lhsT = oh_sb[:, w * P:(w + 1) * P]
                for dh in range(2):
                    nc.tensor.matmul(
                        out=psum_t[:, dh * 512:(dh + 1) * 512],
                        lhsT=lhsT,
                        rhs=feats[:, wi, dh * 512:(dh + 1) * 512],
                        start=(sub == 0),
                        stop=(sub == WPP - 1),
                    )
                if sub == WPP - 1:
                    kk = w // WPP
                    spg = group_sizes[sct_idx]
                    if sct_fill == 0:
                        stage_t = stage_pool.tile(
                            [P, max(group_sizes), D],
                            mybir.dt.bfloat16 if OUT_BF16 else mybir.dt.float32,
                            tag="stage",
                        )
                    if copy_engine == 0:
                        nc.vector.tensor_copy(
                            out=stage_t[:, sct_fill, :], in_=psum_t[:, :])
                    else:
                        nc.scalar.copy(
                            out=stage_t[:, sct_fill, :], in_=psum_t[:, :])
                    copy_engine ^= 1
                    sct_fill += 1
                    if sct_fill == spg:
                        k0 = kk - spg + 1
                        nc.sync.dma_start(
                            out=out_p[:, k0:k0 + spg, :],
                            in_=stage_t[:, 0:spg, :],
                        )
                        sct_fill = 0
                        sct_idx += 1
    nc.compile()
    return nc


def kernel(embed, text_token_ids, text_segments, amr_token_ids, amr_segments):
    global LAST_EXEC_NS, LAST_TRACE
    embed = np.asarray(embed, dtype=np.float32)
    tt = np.asarray(text_token_ids, dtype=np.int32)
    ts_ = np.asarray(text_segments, dtype=np.int32)
    at = np.asarray(amr_token_ids, dtype=np.int32)
    as_ = np.asarray(amr_segments, dtype=np.int32)

    embed_bf16 = np.ascontiguousarray(embed.astype(BF16))

    # --- host-side packing (per core) ---
    per_core = []
    seg_rows_all = []
    for c in range(NCORES):
        tok_rows, seg_rows = [], []
        for r in range(RB):
            b = c * RB + r
            for (tok, seg) in ((at[b], as_[b]), (tt[b], ts_[b])):
                order = np.argsort(seg, kind="stable")
                tok_rows.append(tok[order])
                seg_rows.append(seg[order])
        per_core.append(_prepare_core(tok_rows, seg_rows))
        seg_rows_all.append(seg_rows)

    nwin_max = max(len(w) for w in per_core)
    nwin = ((nwin_max + WPP - 1) // WPP) * WPP

    # static chunk / scatter-group schedule shared by all cores; the last
    # chunks shrink so little work remains after the final gather lands.
    chunk_sizes = []
    rem = nwin - 3 * WPP
    while rem > 0:
        csz = min(CHW, rem)
        chunk_sizes.append(csz)
        rem -= csz
    chunk_sizes += [2 * WPP, WPP]
    npsum = nwin // WPP
    group_sizes = []
    rem = npsum
    while rem > 0:
        g = min(SPG, rem)
        group_sizes.append(g)
        rem -= g

    # --- per-core tensors + compact tables ---
    raw = [_core_tensors(per_core[c], nwin, npsum) for c in range(NCORES)]
    uniqs = []
    for c in range(NCORES):
        gflat, _, _ = raw[c]
        uniqs.append(np.unique(gflat))
    nu = max(len(u) for u in uniqs)

    nc = _build_graph(nwin, chunk_sizes, group_sizes, nu)

    in_maps = []
    for c in range(NCORES):
        gflat, oh, sflat = raw[c]
        uniq = uniqs[c]
        lut = np.zeros(V, dtype=np.int64)
        lut[uniq] = np.arange(len(uniq))
        gcomp = lut[gflat]
        assert gcomp.max() < 32768
        table = np.zeros((nu, D), dtype=BF16)
        table[:len(uniq)] = embed_bf16[uniq]
        in_maps.append({
            "table": table,
            "gidx": _wrap_idx(gcomp),
            "oh": np.ascontiguousarray(oh),
        })

    trace = os.environ.get("KERNEL_TRACE", "") == "1"
    if trace:
        try:
            import axon_hooks_shim
            axon_hooks_shim.install()
        except Exception as e:
            print(f"ntff shim install failed: {e}", file=sys.stderr)
    res = run_bass_kernel_spmd(nc, in_maps, core_ids=list(range(NCORES)),
                               trace=trace)
    LAST_EXEC_NS = res.exec_time_ns
    LAST_TRACE = res

    out = np.zeros((B, 2 * G, D), dtype=np.float32)
    for c in range(NCORES):
        dense = np.asarray(res.results[c]["out"]).astype(np.float32)  # [P, npsum, D]
        _, _, sflat = raw[c]
        islots = np.flatnonzero(sflat != TRASH)
        rows = sflat[islots]
        oc = np.zeros((NROWS, D), dtype=np.float32)
        oc[rows] = dense[islots % P, islots // P]
        oc = oc.reshape(RB, 2, G, D)
        for r in range(RB):
            out[c * RB + r] = oc[r].reshape(2 * G, D)
    return out


# revision 20
# speedup vs baseline: 1.1044x; 1.1044x over previous
"""AMRBART VocabEmbed segment-mean kernel for 8 Trainium2 NeuronCores.

Computes, for two token streams (amr, text):
    feats = embed[token_ids]            # [B, T, D] gather
    means = segment_mean(feats, segs)   # [B, G, D] (empty groups -> 0)
    out   = concat([amr_means, text_means], axis=1)  # [B, 2G, D]

Strategy (data-parallel over batch, no collectives):
  - each of the 8 cores handles B/8 = 2 batch rows x 2 streams.
  - the host packs whole token-groups into 128-token "windows"; a bf16
    matmul onehot[128 tok x 128 slots].T @ feats[128 tok x 1024] produces
    group means directly (one-hot weight = 1/count), accumulated in PSUM
    (2 windows share a PSUM tile via slot bases 0/64).
  - each core's <=16384 tokens hit <16384 distinct vocab rows, so the host
    builds a per-core compacted bf16 table whose indices fit int16 — this
    enables the fast dma_gather/dma_scatter_add (SWDGE "Ant") instructions.
  - outputs are written DENSELY in slot order (full-rate contiguous DMA);
    the host unpermutes slot -> (stream, group) rows afterwards.  The
    compiled graph is identical across cores: all per-core variation is
    carried in input tensors.
"""

import os
import sys
from contextlib import ExitStack

sys.path.insert(0, "/opt/trn_rl_repo")

import numpy as np
import ml_dtypes

from concourse import bacc, bass, mybir
import concourse.tile as tile
from concourse.bass_utils import run_bass_kernel_spmd

BF16 = ml_dtypes.bfloat16

V, D = 50265, 1024
B, T, G = 16, 4096, 1024
NCORES = 8
P = 128                     # SBUF partitions == tokens per window
RB = B // NCORES            # batch rows per core
NRS = RB * 2                # row-streams per core (amr/text per row)
SLOT_CAP = 32               # max group-row span per window
WPP = 4                     # windows per PSUM tile (SLOT_CAP * WPP == P)
CHW = 8                     # windows per gather chunk (1024 idx per
                            # dma_gather; 2048 crashed the runtime)
SPG = 4                     # PSUM tiles per scatter group
NROWS = NRS * G             # real output rows per core
TRASH = NROWS               # extra row absorbing pad-slot zero-adds
OUT_BF16 = os.environ.get("KERNEL_OUT_F32", "") != "1"

# filled by kernel() for test harness introspection
LAST_EXEC_NS = None
LAST_TRACE = None


def _pack_streams(tok_rows, seg_rows):
    """Pack NRS row-streams (already segment-sorted) of one core into
    whole-group windows of <= P tokens and <= SLOT_CAP group-row span.

    Group-rows are global: row = rs * G + g.  Returns a list of windows
    [(tok_ids int32[ntok], cols int32[ntok], wts f32[ntok], row_lo, span)].
    """
    windows = []
    cur_tok = []
    cur_col = []
    cur_wt = []
    cur_lo = None
    cur_hi = None

    def flush():
        nonlocal cur_tok, cur_col, cur_wt, cur_lo, cur_hi
        if cur_lo is None:
            return
        span = cur_hi - cur_lo + 1
        assert span <= SLOT_CAP and len(cur_tok) <= P
        windows.append((
            np.array(cur_tok, dtype=np.int32),
            np.array(cur_col, dtype=np.int32),
            np.array(cur_wt, dtype=np.float32),
            cur_lo, span,
        ))
        cur_tok, cur_col, cur_wt, cur_lo, cur_hi = [], [], [], None, None

    for rs in range(NRS):
        tok, seg = tok_rows[rs], seg_rows[rs]
        bounds = np.flatnonzero(np.diff(seg)) + 1
        starts = np.concatenate(([0], bounds))
        ends = np.concatenate((bounds, [T]))
        counts = ends - starts
        gvals = seg[starts]
        for s, e, n, g in zip(starts, ends, counts, gvals):
            row = rs * G + int(g)
            n = int(n)
            assert n <= P, f"group with {n} > {P} tokens not packable"
            if cur_lo is not None and (
                len(cur_tok) + n > P or row - cur_lo + 1 > SLOT_CAP
            ):
                flush()
            if cur_lo is None:
                cur_lo = row
            cur_hi = row
            w = 1.0 / n
            cur_tok.extend(tok[s:e].tolist())
            col = row - cur_lo
            cur_col.extend([col] * n)
            cur_wt.extend([w] * n)
    flush()
    return windows


def _prepare_core(tok_rows, seg_rows):
    return _pack_streams(tok_rows, seg_rows)


def _wrap_idx(flat):
    """dma_gather/dma_scatter_add index layout: flat index i lives at
    partition i%16, column i//16; the 16-partition block is replicated to
    all 128 partitions.  len(flat) must be a multiple of 16."""
    n = len(flat)
    assert n % 16 == 0
    f = np.asarray(flat, dtype=np.int16).reshape(n // 16, 16).T  # [16, n/16]
    return np.ascontiguousarray(np.tile(f, (8, 1)))              # [128, n/16]


def _core_tensors(windows, nwin, npsum):
    """Build per-core input tensors: compact-id gather idx (wrapped),
    one-hot weights, and the slot -> output-row map used by the host to
    unpermute the densely written output."""
    # flat gather list: slot i = w*128 + p  -> token id (0 pad)
    gflat = np.zeros(nwin * P, dtype=np.int64)
    oh = np.zeros((P, nwin, P), dtype=BF16)
    # flat slot->row map: slot i = k*128 + p -> output row (TRASH pad)
    sflat = np.full(npsum * P, TRASH, dtype=np.int64)

    for w, (tids, cols, wts, row_lo, span) in enumerate(windows):
        ntok = len(tids)
        gflat[w * P:w * P + ntok] = tids
        sub = w % WPP
        base = sub * SLOT_CAP
        oh[np.arange(ntok), w, base + cols] = wts.astype(BF16)
        k = w // WPP
        sflat[k * P + base:k * P + base + span] = row_lo + np.arange(span)
    return gflat, oh, sflat


def _build_graph(nwin, chunk_sizes, group_sizes, nu):
    npsum = nwin // WPP
    out_dt = mybir.dt.bfloat16 if OUT_BF16 else mybir.dt.float32

    nc = bacc.Bacc(num_swdge_queues=2)
    table_p = nc.declare_dram_parameter("table", [nu, D], mybir.dt.bfloat16, False)
    gidx_p = nc.declare_dram_parameter("gidx", [P, nwin * 8], mybir.dt.int16, False)
    oh_p = nc.declare_dram_parameter("oh", [P, nwin, P], mybir.dt.bfloat16, False)
    # dense slot-ordered output: slot (k*128 + p) lives at out[p, k, :]
    out_p = nc.declare_dram_parameter("out", [P, npsum, D], out_dt, True)

    with ExitStack() as ctx:
        tc = ctx.enter_context(tile.TileContext(nc))
        const_pool = ctx.enter_context(tc.tile_pool(name="const", bufs=1))
        feat_pool = ctx.enter_context(tc.tile_pool(name="feats", bufs=3))
        psum_pool = ctx.enter_context(tc.tile_pool(name="psum", bufs=3, space="PSUM"))
        stage_pool = ctx.enter_context(tc.tile_pool(name="stage", bufs=3))

        # gidx loads first: the first gather only needs it (tiny), while the
        # one-hot load (4.5 MB) overlaps the first gather streams.
        gidx_sb = const_pool.tile([P, nwin * 8], mybir.dt.int16)
        nc.sync.dma_start(out=gidx_sb[:], in_=gidx_p[:, :])
        oh_sb = const_pool.tile([P, nwin * P], mybir.dt.bfloat16)
        nc.sync.dma_start(out=oh_sb[:], in_=oh_p.ap().rearrange("p w q -> p (w q)"))

        psum_t = None
        stage_t = None
        sct_idx = 0
        sct_fill = 0
        copy_engine = 0

        for ch, csz in enumerate(chunk_sizes):
            w0 = sum(chunk_sizes[:ch])
            feats = feat_pool.tile([P, csz, D], mybir.dt.bfloat16)
            nc.gpsimd.dma_gather(
                out_ap=feats[:],
                in_ap=table_p[:, :],
                idxs_ap=gidx_sb[:, w0 * 8:(w0 + csz) * 8],
                num_idxs=csz * P,
                num_idxs_reg=csz * P,
                elem_size=D,
                queue_num=ch % 2,
            )
            for wi in range(csz):
                w = w0 + wi
                sub = w % WPP
                if sub == 0:
                    psum_t = psum_pool.tile([P, D], mybir.dt.float32)
                lhsT = oh_sb[:, w * P:(w + 1) * P]
                for dh in range(2):
                    nc.tensor.matmul(
                        out=psum_t[:, dh * 512:(dh + 1) * 512],
                        lhsT=lhsT,
                        rhs=feats[:, wi, dh * 512:(dh + 1) * 512],
                        start=(sub == 0),
                        stop=(sub == WPP - 1),
                    )
                if sub == WPP - 1:
                    kk = w // WPP
                    spg = group_sizes[sct_idx]
                    if sct_fill == 0:
                        stage_t = stage_pool.tile(
                            [P, max(group_sizes), D],
                            mybir.dt.bfloat16 if OUT_BF16 else mybir.dt.float32,
                            tag="stage",
                        )
                    if copy_engine == 0:
                        nc.vector.tensor_copy(
                            out=stage_t[:, sct_fill, :], in_=psum_t[:, :])
                    else:
                        nc.scalar.copy(
                            out=stage_t[:, sct_fill, :], in_=psum_t[:, :])
                    copy_engine ^= 1
                    sct_fill += 1
                    if sct_fill == spg:
                        k0 = kk - spg + 1
                        nc.sync.dma_start(
                            out=out_p[:, k0:k0 + spg, :],
                            in_=stage_t[:, 0:spg, :],
                        )
                        sct_fill = 0
                        sct_idx += 1
    nc.compile()
    return nc


def kernel(embed, text_token_ids, text_segments, amr_token_ids, amr_segments):
    global LAST_EXEC_NS, LAST_TRACE
    embed = np.asarray(embed, dtype=np.float32)
    tt = np.asarray(text_token_ids, dtype=np.int32)
    ts_ = np.asarray(text_segments, dtype=np.int32)
    at = np.asarray(amr_token_ids, dtype=np.int32)
    as_ = np.asarray(amr_segments, dtype=np.int32)

    embed_bf16 = np.ascontiguousarray(embed.astype(BF16))

    # --- host-side packing (per core) ---
    per_core = []
    seg_rows_all = []
    for c in range(NCORES):
        tok_rows, seg_rows = [], []
        for r in range(RB):
            b = c * RB + r
            for (tok, seg) in ((at[b], as_[b]), (tt[b], ts_[b])):
                order = np.argsort(seg, kind="stable")
                tok_rows.append(tok[order])
                seg_rows.append(seg[order])
        per_core.append(_prepare_core(tok_rows, seg_rows))
        seg_rows_all.append(seg_rows)

    nwin_max = max(len(w) for w in per_core)
    nwin = ((nwin_max + WPP - 1) // WPP) * WPP

    # static chunk / scatter-group schedule shared by all cores; the last
    # chunks shrink so little work remains after the final gather lands.
    chunk_sizes = []
    rem = nwin - 3 * WPP
    while rem > 0:
        csz = min(CHW, rem)
        chunk_sizes.append(csz)
        rem -= csz
    chunk_sizes += [2 * WPP, WPP]
    npsum = nwin // WPP
    group_sizes = []
    rem = npsum
    while rem > 0:
        g = min(SPG, rem)
        group_sizes.append(g)
        rem -= g

    # --- per-core tensors + compact tables ---
    raw = [_core_tensors(per_core[c], nwin, npsum) for c in range(NCORES)]
    uniqs = []
    for c in range(NCORES):
        gflat, _, _ = raw[c]
        uniqs.append(np.unique(gflat))
    nu = max(len(u) for u in uniqs)

    nc = _build_graph(nwin, chunk_sizes, group_sizes, nu)

    in_maps = []
    for c in range(NCORES):
        gflat, oh, sflat = raw[c]
        uniq = uniqs[c]
        lut = np.zeros(V, dtype=np.int64)
        lut[uniq] = np.arange(len(uniq))
        gcomp = lut[gflat]
        assert gcomp.max() < 32768
        table = np.zeros((nu, D), dtype=BF16)
        table[:len(uniq)] = embed_bf16[uniq]
        in_maps.append({
            "table": table,
            "gidx": _wrap_idx(gcomp),
            "oh": np.ascontiguousarray(oh),
        })

    trace = os.environ.get("KERNEL_TRACE", "") == "1"
    if trace:
        try:
            import axon_hooks_shim
            axon_hooks_shim.install()
        except Exception as e:
            print(f"ntff shim install failed: {e}", file=sys.stderr)
    res = run_bass_kernel_spmd(nc, in_maps, core_ids=list(range(NCORES)),
                               trace=trace)
    LAST_EXEC_NS = res.exec_time_ns
    LAST_TRACE = res

    out = np.zeros((B, 2 * G, D), dtype=np.float32)
    for c in range(NCORES):
        dense = np.asarray(res.results[c]["out"]).astype(np.float32)  # [P, npsum, D]
        _, _, sflat = raw[c]
        islots = np.flatnonzero(sflat != TRASH)
        rows = sflat[islots]
        oc = np.zeros((NROWS, D), dtype=np.float32)
        oc[rows] = dense[islots % P, islots // P]
        oc = oc.reshape(RB, 2, G, D)
        for r in range(RB):
            out[c * RB + r] = oc[r].reshape(2 * G, D)
    return out


# revision 22
# speedup vs baseline: 1.3043x; 1.1811x over previous
"""AMRBART VocabEmbed segment-mean kernel for 8 Trainium2 NeuronCores.

Computes, for two token streams (amr, text):
    feats = embed[token_ids]            # [B, T, D] gather
    means = segment_mean(feats, segs)   # [B, G, D] (empty groups -> 0)
    out   = concat([amr_means, text_means], axis=1)  # [B, 2G, D]

Strategy (data-parallel over batch, no collectives):
  - each of the 8 cores handles B/8 = 2 batch rows x 2 streams.
  - the host packs whole token-groups into 128-token "windows"; a bf16
    matmul onehot[128 tok x 128 slots].T @ feats[128 tok x 1024] produces
    group means directly (one-hot weight = 1/count), accumulated in PSUM
    (2 windows share a PSUM tile via slot bases 0/64).
  - each core's <=16384 tokens hit <16384 distinct vocab rows, so the host
    builds a per-core compacted bf16 table whose indices fit int16 — this
    enables the fast dma_gather/dma_scatter_add (SWDGE "Ant") instructions.
  - outputs are written DENSELY in slot order (full-rate contiguous DMA);
    the host unpermutes slot -> (stream, group) rows afterwards.  The
    compiled graph is identical across cores: all per-core variation is
    carried in input tensors.
"""

import os
import sys
from contextlib import ExitStack

sys.path.insert(0, "/opt/trn_rl_repo")

import numpy as np
import ml_dtypes

from concourse import bacc, bass, mybir
import concourse.tile as tile
from concourse.bass_utils import run_bass_kernel_spmd

BF16 = ml_dtypes.bfloat16

V, D = 50265, 1024
B, T, G = 16, 4096, 1024
NCORES = 8
P = 128                     # SBUF partitions == tokens per window
RB = B // NCORES            # batch rows per core
NRS = RB * 2                # row-streams per core (amr/text per row)
SLOT_CAP = 32               # max group-row span per window
WPP = 4                     # windows per PSUM tile (SLOT_CAP * WPP == P)
CHW = 8                     # windows per gather chunk (1024 idx per
                            # dma_gather; 2048 crashed the runtime)
SPG = 4                     # PSUM tiles per scatter group
NROWS = NRS * G             # real output rows per core
TRASH = NROWS               # extra row absorbing pad-slot zero-adds
OUT_BF16 = os.environ.get("KERNEL_OUT_F32", "") != "1"

# filled by kernel() for test harness introspection
LAST_EXEC_NS = None
LAST_TRACE = None


def _pack_streams(tok_rows, seg_rows):
    """Pack NRS row-streams (already segment-sorted) of one core into
    whole-group windows of <= P tokens and <= SLOT_CAP group-row span.

    Group-rows are global: row = rs * G + g.  Returns a list of windows
    [(tok_ids int32[ntok], cols int32[ntok], wts f32[ntok], row_lo, span)].
    """
    windows = []
    cur_tok = []
    cur_col = []
    cur_wt = []
    cur_lo = None
    cur_hi = None

    def flush():
        nonlocal cur_tok, cur_col, cur_wt, cur_lo, cur_hi
        if cur_lo is None:
            return
        span = cur_hi - cur_lo + 1
        assert span <= SLOT_CAP and len(cur_tok) <= P
        windows.append((
            np.array(cur_tok, dtype=np.int32),
            np.array(cur_col, dtype=np.int32),
            np.array(cur_wt, dtype=np.float32),
            cur_lo, span,
        ))
        cur_tok, cur_col, cur_wt, cur_lo, cur_hi = [], [], [], None, None

    for rs in range(NRS):
        tok, seg = tok_rows[rs], seg_rows[rs]
        bounds = np.flatnonzero(np.diff(seg)) + 1
        starts = np.concatenate(([0], bounds))
        ends = np.concatenate((bounds, [T]))
        counts = ends - starts
        gvals = seg[starts]
        for s, e, n, g in zip(starts, ends, counts, gvals):
            row = rs * G + int(g)
            n = int(n)
            assert n <= P, f"group with {n} > {P} tokens not packable"
            if cur_lo is not None and (
                len(cur_tok) + n > P or row - cur_lo + 1 > SLOT_CAP
            ):
                flush()
            if cur_lo is None:
                cur_lo = row
            cur_hi = row
            w = 1.0 / n
            cur_tok.extend(tok[s:e].tolist())
            col = row - cur_lo
            cur_col.extend([col] * n)
            cur_wt.extend([w] * n)
    flush()
    return windows


def _prepare_core(tok_rows, seg_rows):
    return _pack_streams(tok_rows, seg_rows)


def _wrap_idx(flat):
    """dma_gather/dma_scatter_add index layout: flat index i lives at
    partition i%16, column i//16; the 16-partition block is replicated to
    all 128 partitions.  len(flat) must be a multiple of 16."""
    n = len(flat)
    assert n % 16 == 0
    f = np.asarray(flat, dtype=np.int16).reshape(n // 16, 16).T  # [16, n/16]
    return np.ascontiguousarray(np.tile(f, (8, 1)))              # [128, n/16]


def _core_tensors(windows, nwin, npsum):
    """Build per-core input tensors: compact-id gather idx (wrapped),
    one-hot weights, and the slot -> output-row map used by the host to
    unpermute the densely written output."""
    # flat gather list: slot i = w*128 + p  -> token id (0 pad)
    gflat = np.zeros(nwin * P, dtype=np.int64)
    oh = np.zeros((P, nwin, P), dtype=BF16)
    # flat slot->row map: slot i = k*128 + p -> output row (TRASH pad)
    sflat = np.full(npsum * P, TRASH, dtype=np.int64)

    for w, (tids, cols, wts, row_lo, span) in enumerate(windows):
        ntok = len(tids)
        gflat[w * P:w * P + ntok] = tids
        sub = w % WPP
        base = sub * SLOT_CAP
        oh[np.arange(ntok), w, base + cols] = wts.astype(BF16)
        k = w // WPP
        sflat[k * P + base:k * P + base + span] = row_lo + np.arange(span)
    return gflat, oh, sflat


def _build_graph(nwin, chunk_sizes, group_sizes, nu):
    npsum = nwin // WPP
    out_dt = mybir.dt.bfloat16 if OUT_BF16 else mybir.dt.float32

    nc = bacc.Bacc(num_swdge_queues=2)
    table_p = nc.declare_dram_parameter("table", [nu, D], mybir.dt.bfloat16, False)
    gidx_p = nc.declare_dram_parameter("gidx", [P, nwin * 8], mybir.dt.int16, False)
    oh_p = nc.declare_dram_parameter("oh", [P, nwin, P], mybir.dt.bfloat16, False)
    # dense slot-ordered output: slot (k*128 + p) lives at out[p, k, :]
    out_p = nc.declare_dram_parameter("out", [P, npsum, D], out_dt, True)

    with ExitStack() as ctx:
        tc = ctx.enter_context(tile.TileContext(nc))
        const_pool = ctx.enter_context(tc.tile_pool(name="const", bufs=1))
        feat_pool = ctx.enter_context(tc.tile_pool(name="feats", bufs=4))
        psum_pool = ctx.enter_context(tc.tile_pool(name="psum", bufs=4, space="PSUM"))
        stage_pool = ctx.enter_context(tc.tile_pool(name="stage", bufs=4))

        # gidx loads first on the sync HWDGE ring (tiny, unblocks gathers);
        # the 4.5 MB one-hot load goes on the scalar engine's separate
        # HWDGE ring so it can't queue ahead of anything gather-critical.
        gidx_sb = const_pool.tile([P, nwin * 8], mybir.dt.int16)
        nc.sync.dma_start(out=gidx_sb[:], in_=gidx_p[:, :])
        oh_sb = const_pool.tile([P, nwin * P], mybir.dt.bfloat16)
        nc.scalar.dma_start(out=oh_sb[:], in_=oh_p.ap().rearrange("p w q -> p (w q)"))

        psum_t = None
        stage_t = None
        sct_idx = 0
        sct_fill = 0
        copy_engine = 0

        for ch, csz in enumerate(chunk_sizes):
            w0 = sum(chunk_sizes[:ch])
            feats = feat_pool.tile([P, csz, D], mybir.dt.bfloat16)
            nc.gpsimd.dma_gather(
                out_ap=feats[:],
                in_ap=table_p[:, :],
                idxs_ap=gidx_sb[:, w0 * 8:(w0 + csz) * 8],
                num_idxs=csz * P,
                num_idxs_reg=csz * P,
                elem_size=D,
                queue_num=ch % 2,
            )
            for wi in range(csz):
                w = w0 + wi
                sub = w % WPP
                if sub == 0:
                    psum_t = psum_pool.tile([P, D], mybir.dt.float32)
                lhsT = oh_sb[:, w * P:(w + 1) * P]
                for dh in range(2):
                    nc.tensor.matmul(
                        out=psum_t[:, dh * 512:(dh + 1) * 512],
                        lhsT=lhsT,
                        rhs=feats[:, wi, dh * 512:(dh + 1) * 512],
                        start=(sub == 0),
                        stop=(sub == WPP - 1),
                    )
                if sub == WPP - 1:
                    kk = w // WPP
                    spg = group_sizes[sct_idx]
                    if sct_fill == 0:
                        stage_t = stage_pool.tile(
                            [P, max(group_sizes), D],
                            mybir.dt.bfloat16 if OUT_BF16 else mybir.dt.float32,
                            tag="stage",
                        )
                    if copy_engine == 0:
                        nc.vector.tensor_copy(
                            out=stage_t[:, sct_fill, :], in_=psum_t[:, :])
                    else:
                        nc.scalar.copy(
                            out=stage_t[:, sct_fill, :], in_=psum_t[:, :])
                    copy_engine ^= 1
                    sct_fill += 1
                    if sct_fill == spg:
                        k0 = kk - spg + 1
                        nc.sync.dma_start(
                            out=out_p[:, k0:k0 + spg, :],
                            in_=stage_t[:, 0:spg, :],
                        )
                        sct_fill = 0
                        sct_idx += 1
    nc.compile()
    return nc


def kernel(embed, text_token_ids, text_segments, amr_token_ids, amr_segments):
    global LAST_EXEC_NS, LAST_TRACE
    embed = np.asarray(embed, dtype=np.float32)
    tt = np.asarray(text_token_ids, dtype=np.int32)
    ts_ = np.asarray(text_segments, dtype=np.int32)
    at = np.asarray(amr_token_ids, dtype=np.int32)
    as_ = np.asarray(amr_segments, dtype=np.int32)

    embed_bf16 = np.ascontiguousarray(embed.astype(BF16))

    # --- host-side packing (per core) ---
    per_core = []
    seg_rows_all = []
    for c in range(NCORES):
        tok_rows, seg_rows = [], []
        for r in range(RB):
            b = c * RB + r
            for (tok, seg) in ((at[b], as_[b]), (tt[b], ts_[b])):
                order = np.argsort(seg, kind="stable")
                tok_rows.append(tok[order])
                seg_rows.append(seg[order])
        per_core.append(_prepare_core(tok_rows, seg_rows))
        seg_rows_all.append(seg_rows)

    nwin_max = max(len(w) for w in per_core)
    nwin = ((nwin_max + WPP - 1) // WPP) * WPP

    # static chunk / scatter-group schedule shared by all cores; the last
    # chunks shrink so little work remains after the final gather lands.
    chunk_sizes = []
    rem = nwin - 3 * WPP
    while rem > 0:
        csz = min(CHW, rem)
        chunk_sizes.append(csz)
        rem -= csz
    chunk_sizes += [2 * WPP, WPP]
    npsum = nwin // WPP
    group_sizes = []
    rem = npsum
    while rem > 0:
        g = min(SPG, rem)
        group_sizes.append(g)
        rem -= g

    # --- per-core tensors + compact tables ---
    raw = [_core_tensors(per_core[c], nwin, npsum) for c in range(NCORES)]
    uniqs = []
    for c in range(NCORES):
        gflat, _, _ = raw[c]
        uniqs.append(np.unique(gflat))
    nu = max(len(u) for u in uniqs)

    nc = _build_graph(nwin, chunk_sizes, group_sizes, nu)

    in_maps = []
    for c in range(NCORES):
        gflat, oh, sflat = raw[c]
        uniq = uniqs[c]
        lut = np.zeros(V, dtype=np.int64)
        lut[uniq] = np.arange(len(uniq))
        gcomp = lut[gflat]
        assert gcomp.max() < 32768
        table = np.zeros((nu, D), dtype=BF16)
        table[:len(uniq)] = embed_bf16[uniq]
        in_maps.append({
            "table": table,
            "gidx": _wrap_idx(gcomp),
            "oh": np.ascontiguousarray(oh),
        })

    trace = os.environ.get("KERNEL_TRACE", "") == "1"
    if trace:
        try:
            import axon_hooks_shim
            axon_hooks_shim.install()
        except Exception as e:
            print(f"ntff shim install failed: {e}", file=sys.stderr)
    res = run_bass_kernel_spmd(nc, in_maps, core_ids=list(range(NCORES)),
                               trace=trace)
    LAST_EXEC_NS = res.exec_time_ns
    LAST_TRACE = res

    out = np.zeros((B, 2 * G, D), dtype=np.float32)
    for c in range(NCORES):
        dense = np.asarray(res.results[c]["out"]).astype(np.float32)  # [P, npsum, D]
        _, _, sflat = raw[c]
        islots = np.flatnonzero(sflat != TRASH)
        rows = sflat[islots]
        oc = np.zeros((NROWS, D), dtype=np.float32)
        oc[rows] = dense[islots % P, islots // P]
        oc = oc.reshape(RB, 2, G, D)
        for r in range(RB):
            out[c * RB + r] = oc[r].reshape(2 * G, D)
    return out
